# revision 1
# baseline (speedup 1.0000x reference)
"""GNN message-passing (segment-mean + linear + relu) Trainium2 kernel.

Sharding: the batch's unique seed nodes are partitioned across 8 cores
round-robin over the sorted unique-node list (so each core's node ids span
all int16 index chunks evenly); edges are colocated with their source
node's core, and only edges whose source is a seed node are kept (the rest
cannot affect the output). Features are replicated in HBM on every core
(the "halo exchange for remote dst features" degenerates to replication
since dst is uniform over all nodes).

Per-core device algorithm:
  phase 1: for each 128-slot block of unique nodes, gather features[dst]
    for the block's edges (dma_gather Q7 ucode; int16 indices, so dsts are
    bucketed into node-id chunks of 32768 and gathers issued per chunk,
    each split into <=1024-index pieces for the SWDGE descriptor ring),
    build one-hot edge->slot matrices on the DVE (one batched is_equal of
    replicated seg values vs an iota row per gather), and accumulate
    sum_t G_t^T @ S_t into PSUM [feat, slot] on the PE.
  phase 2: per block, PE-transpose the gathered self features, matmul the
    neighbor sums against W2^T and scale by 1/deg (per-partition scalar on
    ACT), matmul self features against W1^T, add (+bias) + relu, DMA out.

Output: [U_cap, 128] rows per core = outputs for that core's unique nodes;
the host scatters rows back to the [50000, 128] batch (duplicate seed
nodes share identical output rows by construction).
"""

import sys

for _p in ("/opt/trn_rl_repo",):
    if _p not in sys.path:
        sys.path.insert(0, _p)

import numpy as np

import concourse.bacc as bacc
import concourse.bass as bass
import concourse.mybir as mybir
from concourse.library_config import mlp
from concourse.tile import TileContext

P = 128
CHUNK = 32768  # int16 index range for dma_gather


def _roundup(x, m):
    return (x + m - 1) // m * m


def _wrap16(vals, dtype=np.int16):
    """dma_gather index layout: wrapped[p, s] = vals[s*16 + (p % 16)],
    replicated across all 128 partitions."""
    vals = np.asarray(vals)
    n = vals.shape[0]
    assert n % 16 == 0
    w = vals.reshape(n // 16, 16).T.astype(dtype)  # [16, n/16]
    return np.tile(w, (8, 1))  # [128, n/16]


def preprocess(nodes, features, edge_index, W, b, n_cores=8, nbg_blocks=4):
    """Host-side index-space preprocessing. Returns (plan, in_maps, assemble)
    where assemble(core_outputs) -> full [B, D] output."""
    nodes = np.asarray(nodes).astype(np.int64)
    features = np.ascontiguousarray(np.asarray(features, dtype=np.float32))
    src = np.asarray(edge_index[0]).astype(np.int64)
    dst = np.asarray(edge_index[1]).astype(np.int64)
    W = np.asarray(W, dtype=np.float32)
    b = np.asarray(b, dtype=np.float32)

    N, D = features.shape
    assert D == 128 and W.shape == (D, 2 * D)
    nchunk = (N + CHUNK - 1) // CHUNK

    features_h = features.astype(np.float16)
    uniq, inv = np.unique(nodes, return_inverse=True)
    U = len(uniq)
    deg = np.bincount(src, minlength=N).astype(np.float64)

    # interleave unique nodes across cores so every core's node-id set spans
    # all int16 chunks roughly evenly (uniq is sorted: a contiguous slice
    # would concentrate in one chunk)
    core_of = np.arange(U) % n_cores
    core_idx = [np.arange(c, U, n_cores) for c in range(n_cores)]
    pos = np.zeros(U, dtype=np.int64)
    chunk_counts = np.zeros((n_cores, nchunk), dtype=np.int64)
    rank_in_run = [None] * n_cores
    chunk_of = [None] * n_cores
    for c in range(n_cores):
        nds = uniq[core_idx[c]]
        ch = nds // CHUNK  # non-decreasing
        run_starts = np.searchsorted(ch, np.arange(nchunk))
        chunk_counts[c] = np.searchsorted(ch, np.arange(nchunk) + 1) - run_starts
        rank_in_run[c] = np.arange(len(nds)) - run_starts[ch]
        chunk_of[c] = ch
    CSELF_CAP = max(128, _roundup(int(chunk_counts.max()), P))
    U_cap = nchunk * CSELF_CAP
    NBLK = U_cap // P
    NBLK_pad = _roundup(NBLK, nbg_blocks)
    NBG = NBLK_pad // nbg_blocks

    for c in range(n_cores):
        pos[core_idx[c]] = rank_in_run[c] + chunk_of[c] * CSELF_CAP

    # per-core slot tables
    slot_node = np.zeros((n_cores, U_cap), dtype=np.int64)
    slot_invdeg = np.zeros((n_cores, U_cap), dtype=np.float32)
    for c in range(n_cores):
        ci = core_idx[c]
        slot_node[c, pos[ci]] = uniq[ci]
        slot_invdeg[c, pos[ci]] = (1.0 / np.maximum(deg[uniq[ci]], 1.0)).astype(
            np.float32
        )

    # edges: keep only those whose src is a seed node
    upos_of_node = np.full(N, -1, dtype=np.int64)
    upos_of_node[uniq] = np.arange(U)
    eu = upos_of_node[src]
    keep = eu >= 0
    eu = eu[keep]
    ed = dst[keep]
    ecore = core_of[eu]
    epos = pos[eu]
    eblock = epos // P
    echunk = ed // CHUNK

    # per (core, block, chunk) counts -> shared tile counts T[b, k]
    flat = (ecore * NBLK_pad + eblock) * nchunk + echunk
    cnt = np.bincount(flat, minlength=n_cores * NBLK_pad * nchunk).reshape(
        n_cores, NBLK_pad, nchunk
    )
    T = np.ceil(cnt.max(axis=0) / P).astype(np.int64)  # [NBLK_pad, nchunk]
    # blocks that are padding on EVERY core (tail of each chunk's slot run
    # beyond the max real count, plus NBLK_pad rounding): no core has a real
    # slot there, so skip all work and never write their output rows
    real_csb = np.maximum(
        1, -(-chunk_counts.max(axis=0) // P)
    )  # [nchunk] blocks actually used per chunk run
    CSB_ = CSELF_CAP // P
    skip_blocks = set()
    for k in range(nchunk):
        for bloc in range(int(real_csb[k]), CSB_):
            skip_blocks.add(k * CSB_ + bloc)
    for blk in range(NBLK, NBLK_pad):
        skip_blocks.add(blk)
    T[sorted(skip_blocks), :] = 0
    # every remaining block needs >= 1 tile so its PSUM region is written
    empty = (T.sum(axis=1) == 0) & ~np.isin(np.arange(NBLK_pad), sorted(skip_blocks))
    T[empty, 0] = 1

    # per-gather capacities and offsets
    EG_CAP = np.zeros((NBG, nchunk), dtype=np.int64)  # num_idxs per gather
    for g in range(NBG):
        for k in range(nchunk):
            EG_CAP[g, k] = P * int(T[g * nbg_blocks : (g + 1) * nbg_blocks, k].sum())
    idx_off = np.zeros((NBG, nchunk), dtype=np.int64)  # offset into idx array /16
    acc = 0
    for g in range(NBG):
        for k in range(nchunk):
            idx_off[g, k] = acc
            acc += EG_CAP[g, k] // 16
    IDX_COLS = int(acc)

    # seg columns: global tile order (g, k, b, t)
    col0 = np.zeros((NBG, nchunk), dtype=np.int64)
    acc = 0
    for g in range(NBG):
        for k in range(nchunk):
            col0[g, k] = acc
            acc += EG_CAP[g, k] // P
    T_TOTAL = int(acc)

    # build per-core arrays
    in_maps = []
    for c in range(n_cores):
        m = ecore == c
        ceb, cek, ced, cep = eblock[m], echunk[m], ed[m], epos[m]
        order = np.lexsort((ced, cek, ceb))
        ceb, cek, ced, cep = ceb[order], cek[order], ced[order], cep[order]
        # group boundaries per (block, chunk)
        key = ceb * nchunk + cek
        bc_cnt = np.bincount(key, minlength=NBLK_pad * nchunk).reshape(
            NBLK_pad, nchunk
        )
        flatc = np.concatenate([[0], np.cumsum(bc_cnt.reshape(-1))[:-1]])
        starts = flatc.reshape(NBLK_pad, nchunk)

        edge_idx_vals = np.zeros(IDX_COLS * 16, dtype=np.int64)
        seg = np.full((P, T_TOTAL), -1.0, dtype=np.float16)
        for g in range(NBG):
            for k in range(nchunk):
                base_i = idx_off[g, k] * 16
                base_t = col0[g, k]
                off = 0
                for bb in range(nbg_blocks):
                    blk = g * nbg_blocks + bb
                    tcount = int(T[blk, k])
                    if tcount == 0:
                        continue
                    n = int(bc_cnt[blk, k])
                    s0 = int(starts[blk, k])
                    # idx values: dst - k*CHUNK (pad -> 0)
                    vals = np.zeros(tcount * P, dtype=np.int64)
                    vals[:n] = ced[s0 : s0 + n] - k * CHUNK
                    edge_idx_vals[base_i + off * P : base_i + (off + tcount) * P] = (
                        vals
                    )
                    # seg values: slot within block (pad -> -1)
                    sv = np.full(tcount * P, -1.0, dtype=np.float16)
                    sv[:n] = (cep[s0 : s0 + n] - blk * P).astype(np.float32)
                    seg[:, base_t + off : base_t + off + tcount] = sv.reshape(
                        tcount, P
                    ).T
                    off += tcount
        assert np.all(edge_idx_vals >= 0) and np.all(edge_idx_vals < CHUNK)

        # self-gather indices: position q -> chunk q // CSELF_CAP
        q = np.arange(U_cap)
        real = np.zeros(U_cap, dtype=bool)
        real[pos[core_idx[c]]] = True
        self_vals = np.where(real, slot_node[c] - (q // CSELF_CAP) * CHUNK, 0)
        assert np.all(self_vals >= 0) and np.all(self_vals < CHUNK)

        invw = slot_invdeg[c].reshape(NBLK, P).T.copy()  # [128, NBLK]
        if NBLK_pad > NBLK:
            invw = np.concatenate(
                [invw, np.zeros((P, NBLK_pad - NBLK), np.float32)], axis=1
            )

        in_maps.append(
            {
                "features": features,
                "features_h": features_h,
                "edge_idx": _wrap16(edge_idx_vals),
                "self_idx": _wrap16(self_vals),
                "seg": seg,
                "invdeg": invw,
                "w1t": W[:, :D].T.copy(),
                "w2t": W[:, D:].T.copy(),
                "bias_bc": np.tile(b, (P, 1)),
                "iota": np.tile(np.arange(P, dtype=np.float16), (P, 1)),
                "ident": np.eye(P, dtype=np.float32),
            }
        )

    plan = {
        "N": N,
        "D": D,
        "nchunk": nchunk,
        "CSELF_CAP": CSELF_CAP,
        "U_cap": U_cap,
        "NBLK": NBLK,
        "NBLK_pad": NBLK_pad,
        "NBG": NBG,
        "nbg_blocks": nbg_blocks,
        "T": T,
        "EG_CAP": EG_CAP,
        "idx_off": idx_off,
        "col0": col0,
        "IDX_COLS": IDX_COLS,
        "T_TOTAL": T_TOTAL,
        "n_cores": n_cores,
        "bias_nonzero": bool(np.any(b != 0)),
        "skip_blocks": skip_blocks,
        "real_csb": real_csb,
    }

    out_core = core_of[inv]
    out_pos = pos[inv]

    def assemble(core_outputs):
        stacked = np.stack(core_outputs)  # [n_cores, U_cap_pad, D]
        return np.ascontiguousarray(stacked[out_core, out_pos])

    return plan, in_maps, assemble


def build_kernel(plan, reps=1, ge_bufs=8, s_bufs=8, blk_bufs=4, p1_bufs=2, p2_bufs=2, ni_tiles=8):
    N, D = plan["N"], plan["D"]
    nchunk = plan["nchunk"]
    CSELF_CAP = plan["CSELF_CAP"]
    U_cap = plan["U_cap"]
    NBLK_pad = plan["NBLK_pad"]
    NBG = plan["NBG"]
    nbg_blocks = plan["nbg_blocks"]
    T = plan["T"]
    EG_CAP = plan["EG_CAP"]
    idx_off = plan["idx_off"]
    col0 = plan["col0"]
    IDX_COLS = plan["IDX_COLS"]
    T_TOTAL = plan["T_TOTAL"]
    CSB = CSELF_CAP // P
    EG_TILES_MAX = int(EG_CAP.max()) // P

    f32 = mybir.dt.float32
    NQ = 4  # SWDGE queues, round-robin
    # descriptor-ring capacity caps one dma_gather at ~1024 indices
    NI_TILES = ni_tiles
    nc = bacc.Bacc("TRN2", target_bir_lowering=False, num_swdge_queues=NQ)
    # one reusable Pool-engine register for dma_gather valid-index counts
    cnt_reg = list(
        nc.alloc_registers("gather_cnt", engines=[mybir.EngineType.Pool])
    )[0]
    qrr = [0]

    def emit_gather(out3d, tile0, ntiles, in_ap, idx_tile, idxcol0):
        """dma_gather split into <=NI_TILES*128-index pieces."""
        for p0 in range(0, ntiles, NI_TILES):
            p1 = min(p0 + NI_TILES, ntiles)
            ni = (p1 - p0) * P
            nc.gpsimd.reg_mov(cnt_reg, ni)
            nc.gpsimd.dma_gather(
                out_ap=out3d[:, tile0 + p0 : tile0 + p1, :],
                in_ap=in_ap,
                idxs_ap=idx_tile[:, idxcol0 + p0 * 8 : idxcol0 + p1 * 8],
                num_idxs=ni,
                num_idxs_reg=cnt_reg,
                elem_size=D,
                queue_num=qrr[0] % NQ,
            )
            qrr[0] += 1

    f16 = mybir.dt.float16
    feat = nc.dram_tensor("features", [N, D], f32, kind="ExternalInput")
    feat_h = nc.dram_tensor("features_h", [N, D], f16, kind="ExternalInput")
    edge_idx_d = nc.dram_tensor(
        "edge_idx", [P, IDX_COLS], mybir.dt.int16, kind="ExternalInput"
    )
    self_idx_d = nc.dram_tensor(
        "self_idx", [P, U_cap // 16], mybir.dt.int16, kind="ExternalInput"
    )
    seg_d = nc.dram_tensor("seg", [P, T_TOTAL], f16, kind="ExternalInput")
    invdeg_d = nc.dram_tensor("invdeg", [P, NBLK_pad], f32, kind="ExternalInput")
    w1t_d = nc.dram_tensor("w1t", [D, D], f32, kind="ExternalInput")
    w2t_d = nc.dram_tensor("w2t", [D, D], f32, kind="ExternalInput")
    bias_d = nc.dram_tensor("bias_bc", [P, D], f32, kind="ExternalInput")
    iota_d = nc.dram_tensor("iota", [P, P], f16, kind="ExternalInput")
    ident_d = nc.dram_tensor("ident", [P, P], f32, kind="ExternalInput")
    out_d = nc.dram_tensor(
        "out", [NBLK_pad * P, D], f32, kind="ExternalOutput"
    )

    with TileContext(nc) as tc:
        with (
            tc.tile_pool(name="const", bufs=1) as const_pool,
            tc.tile_pool(name="gself", bufs=1) as gself_pool,
            tc.tile_pool(name="ge", bufs=ge_bufs) as ge_pool,
            tc.tile_pool(name="s", bufs=s_bufs) as s_pool,
            tc.tile_pool(name="blk", bufs=blk_bufs) as blk_pool,
            tc.tile_pool(name="psum1", bufs=p1_bufs, space="PSUM") as psum1_pool,
            tc.tile_pool(name="psum2", bufs=p2_bufs, space="PSUM") as psum2_pool,
        ):
            nc.gpsimd.load_library(mlp)

            def load_const(dram, shape, dtype=f32, tag=None):
                t = const_pool.tile(shape, dtype, tag=tag)
                nc.sync.dma_start(t[:], dram[:])
                return t

            edge_idx = load_const(
                edge_idx_d, [P, IDX_COLS], mybir.dt.int16, tag="edge_idx"
            )
            self_idx = load_const(
                self_idx_d, [P, U_cap // 16], mybir.dt.int16, tag="self_idx"
            )
            seg = load_const(seg_d, [P, T_TOTAL], f16, tag="seg")
            invdeg = load_const(invdeg_d, [P, NBLK_pad], tag="invdeg")
            w1t = load_const(w1t_d, [D, D], tag="w1t")
            w2t = load_const(w2t_d, [D, D], tag="w2t")
            bias_bc = load_const(bias_d, [P, D], tag="bias_bc")
            iota = load_const(iota_d, [P, P], f16, tag="iota")
            ident = load_const(ident_d, [P, P], tag="ident")

            for _rep in range(reps):
                # self features for all slots: [slot%128, slot//128, feat]
                gself = gself_pool.tile([P, NBLK_pad, D], f32)
                for k in range(nchunk):
                    emit_gather(
                        gself, k * CSB, int(plan["real_csb"][k]),
                        feat[k * CHUNK :, :],
                        self_idx, k * (CSELF_CAP // 16),
                    )
                if NBLK_pad > U_cap // P:
                    nc.vector.memset(gself[:, U_cap // P :, :], 0.0)

                for g in range(NBG):
                    gbuf = {}
                    stile = {}
                    for k in range(nchunk):
                        tgk = int(EG_CAP[g, k]) // P
                        if tgk == 0:
                            continue
                        gb = ge_pool.tile([P, EG_TILES_MAX, D], f16, tag="ge")
                        emit_gather(
                            gb, 0, tgk,
                            feat_h[k * CHUNK :, :],
                            edge_idx, int(idx_off[g, k]),
                        )
                        gbuf[k] = gb
                        # batched one-hot: S[p, t, w] = (seg[p, col0+t] == iota[w])
                        st = s_pool.tile([P, EG_TILES_MAX, P], f16, tag="s")
                        c0 = int(col0[g, k])
                        seg_rep = seg[:, c0 : c0 + tgk].rearrange(
                            "p (t o) -> p t o", o=1
                        ).to_broadcast([P, tgk, P])
                        iota_rep = iota[:, :].rearrange(
                            "p (o w) -> p o w", o=1
                        ).to_broadcast([P, tgk, P])
                        nc.vector.tensor_tensor(
                            out=st[:, :tgk, :],
                            in0=seg_rep,
                            in1=iota_rep,
                            op=mybir.AluOpType.is_equal,
                        )
                        stile[k] = st

                    psum1 = psum1_pool.tile([P, nbg_blocks, P], f32, tag="p1")
                    # per-block static schedule of (chunk, local tile) pairs
                    sched = [[] for _ in range(nbg_blocks)]
                    for k in range(nchunk):
                        off = 0
                        for bb in range(nbg_blocks):
                            tcount = int(T[g * nbg_blocks + bb, k])
                            for t in range(tcount):
                                sched[bb].append((k, off + t))
                            off += tcount
                    # block-major: one PSUM accumulation group open at a time
                    # (PSUM zero-region = full bank; groups can't interleave)
                    for bb in range(nbg_blocks):
                        total = len(sched[bb])
                        for i, (k, t) in enumerate(sched[bb]):
                            nc.tensor.matmul(
                                out=psum1[:, bb, :],
                                lhsT=gbuf[k][:, t, :],
                                rhs=stile[k][:, t, :],
                                start=(i == 0),
                                stop=(i == total - 1),
                            )

                    for bb in range(nbg_blocks):
                        blk = g * nbg_blocks + bb
                        if blk in plan["skip_blocks"]:
                            continue
                        msum_t = blk_pool.tile([P, P], f32, tag="msumT")
                        nc.scalar.activation(
                            msum_t[:], psum1[:, bb, :], mybir.ActivationFunctionType.Copy
                        )
                        psum_tr = psum2_pool.tile([P, P], f32, tag="ptr")
                        nc.tensor.transpose(psum_tr[:], gself[:, blk, :], ident[:])
                        self_t = blk_pool.tile([P, P], f32, tag="selfT")
                        nc.scalar.activation(
                            self_t[:], psum_tr[:], mybir.ActivationFunctionType.Copy
                        )
                        psum_a = psum2_pool.tile([P, P], f32, tag="pa")
                        nc.tensor.matmul(
                            out=psum_a[:], lhsT=msum_t[:], rhs=w2t[:], start=True, stop=True
                        )
                        z2 = blk_pool.tile([P, P], f32, tag="z2")
                        nc.scalar.activation(
                            z2[:],
                            psum_a[:],
                            mybir.ActivationFunctionType.Copy,
                            scale=invdeg[:, blk : blk + 1],
                        )
                        psum_b = psum2_pool.tile([P, P], f32, tag="pb")
                        nc.tensor.matmul(
                            out=psum_b[:], lhsT=self_t[:], rhs=w1t[:], start=True, stop=True
                        )
                        o1 = blk_pool.tile([P, P], f32, tag="o1")
                        nc.vector.tensor_tensor(
                            out=o1[:], in0=psum_b[:], in1=z2[:], op=mybir.AluOpType.add
                        )
                        if plan["bias_nonzero"]:
                            nc.vector.tensor_tensor(
                                out=o1[:], in0=o1[:], in1=bias_bc[:], op=mybir.AluOpType.add
                            )
                        out_sb = blk_pool.tile([P, P], f32, tag="osb")
                        nc.scalar.activation(
                            out_sb[:], o1[:], mybir.ActivationFunctionType.Relu
                        )
                        nc.sync.dma_start(out_d[blk * P : (blk + 1) * P, :], out_sb[:])

    nc.compile()
    return nc


_RUN_KWARGS = {}


def run_on_hw(nc, in_maps, n_cores, **kwargs):
    from concourse.bass_utils import run_bass_kernel_spmd

    return run_bass_kernel_spmd(nc, in_maps, list(range(n_cores)), **kwargs)


def kernel(nodes, features, edge_index, W, b):
    """Full-input entry point: shards internally across 8 NeuronCores."""
    n_cores = 8
    plan, in_maps, assemble = preprocess(
        nodes, features, edge_index, W, b, n_cores=n_cores
    )
    nc = build_kernel(plan)
    res = run_on_hw(nc, in_maps, n_cores, **_RUN_KWARGS)
    outs = [np.asarray(r["out"]) for r in res.results]
    return np.ascontiguousarray(assemble(outs).astype(np.float32))



# revision 2
# speedup vs baseline: 4.8645x; 4.8645x over previous
"""GNN message-passing (segment-mean + linear + relu) Trainium2 kernel, v2.

Differences vs v1 (kernel.py):
  - edges packed contiguously per (group=4 blocks, chunk) with trailing -1
    index padding (ucode skips trailing negatives -> fewer descriptors);
    tiles can straddle block boundaries, handled by per-(tile,block) one-hot
    columns (zero rows for out-of-block edges).
  - self features gathered with transpose=True (f16) directly into
    [feat, slot] layout -> phase 2 needs no PE transpose.
  - phase 2 fused: neighbor-mean scaling via DVE multiply with a
    [128, U_cap] inverse-degree row constant, then BOTH linear matmuls
    accumulate into one PSUM bank; ACT applies relu; DMA out.
  - deeper gather lookahead (ge_bufs) so the SWDGE gather stream runs
    continuously instead of stalling per group.
"""

import sys

for _p in ("/opt/trn_rl_repo",):
    if _p not in sys.path:
        sys.path.insert(0, _p)

import numpy as np

import concourse.bacc as bacc
import concourse.bass as bass
import concourse.mybir as mybir
from concourse.library_config import mlp
from concourse.tile import TileContext

P = 128
CHUNK = 32768  # int16 index range for dma_gather
GBLK = 4  # blocks per group


def _roundup(x, m):
    return (x + m - 1) // m * m


def _wrap16(vals, dtype=np.int16):
    vals = np.asarray(vals)
    n = vals.shape[0]
    assert n % 16 == 0
    w = vals.reshape(n // 16, 16).T.astype(dtype)  # [16, n/16]
    return np.tile(w, (8, 1))  # [128, n/16]


def preprocess(nodes, features, edge_index, W, b, n_cores=8, negpad=False):
    nodes = np.asarray(nodes).astype(np.int64)
    features = np.ascontiguousarray(np.asarray(features, dtype=np.float32))
    src = np.asarray(edge_index[0]).astype(np.int64)
    dst = np.asarray(edge_index[1]).astype(np.int64)
    W = np.asarray(W, dtype=np.float32)
    b = np.asarray(b, dtype=np.float32)

    N, D = features.shape
    assert D == 128 and W.shape == (D, 2 * D)
    nchunk = (N + CHUNK - 1) // CHUNK

    features_h = features.astype(np.float16)
    uniq, inv = np.unique(nodes, return_inverse=True)
    U = len(uniq)
    deg = np.bincount(src, minlength=N).astype(np.float64)

    # interleave unique nodes across cores; within a core, slots are laid out
    # in runs per int16 chunk (slot q -> chunk q // CSELF_CAP)
    core_of = np.arange(U) % n_cores
    core_idx = [np.arange(c, U, n_cores) for c in range(n_cores)]
    pos = np.zeros(U, dtype=np.int64)  # rank within chunk run, for now
    chunk_counts = np.zeros((n_cores, nchunk), dtype=np.int64)
    for c in range(n_cores):
        nds = uniq[core_idx[c]]
        ch = nds // CHUNK  # non-decreasing
        run_starts = np.searchsorted(ch, np.arange(nchunk))
        chunk_counts[c] = np.searchsorted(ch, np.arange(nchunk) + 1) - run_starts
        pos[core_idx[c]] = np.arange(len(nds)) - run_starts[ch]
    CSELF_CAP = max(P, _roundup(int(chunk_counts.max()), P))
    U_cap = nchunk * CSELF_CAP
    NBLK = U_cap // P
    NBLK_pad = _roundup(NBLK, GBLK)
    NBG = NBLK_pad // GBLK

    chunk_of_node = uniq // CHUNK
    for c in range(n_cores):
        ci = core_idx[c]
        pos[ci] = pos[ci] + chunk_of_node[ci] * CSELF_CAP

    # per-core slot tables
    slot_node = np.zeros((n_cores, U_cap), dtype=np.int64)
    slot_invdeg = np.zeros((n_cores, U_cap), dtype=np.float32)
    slot_real = np.zeros((n_cores, U_cap), dtype=bool)
    for c in range(n_cores):
        ci = core_idx[c]
        slot_node[c, pos[ci]] = uniq[ci]
        slot_invdeg[c, pos[ci]] = (1.0 / np.maximum(deg[uniq[ci]], 1.0)).astype(
            np.float32
        )
        slot_real[c, pos[ci]] = True

    # blocks that are padding on EVERY core: tail of each chunk's slot run
    # beyond the max real count, plus NBLK->NBLK_pad rounding
    real_csb = np.maximum(1, -(-chunk_counts.max(axis=0) // P))  # [nchunk]
    CSB = CSELF_CAP // P
    skip_blocks = set()
    for k in range(nchunk):
        for bloc in range(int(real_csb[k]), CSB):
            skip_blocks.add(k * CSB + bloc)
    for blk in range(NBLK, NBLK_pad):
        skip_blocks.add(blk)

    # edges: keep only those whose src is a seed node
    upos_of_node = np.full(N, -1, dtype=np.int64)
    upos_of_node[uniq] = np.arange(U)
    eu = upos_of_node[src]
    keep = eu >= 0
    eu = eu[keep]
    ed = dst[keep]
    ecore = core_of[eu]
    epos = pos[eu]
    eblock = epos // P
    egrp = eblock // GBLK
    echunk = ed // CHUNK

    # per (core, group, chunk) counts -> shared tile counts T[g, k]
    flat = (ecore * NBG + egrp) * nchunk + echunk
    cnt = np.bincount(flat, minlength=n_cores * NBG * nchunk).reshape(
        n_cores, NBG, nchunk
    )
    T = np.ceil(cnt.max(axis=0) / P).astype(np.int64)  # [NBG, nchunk]

    # shared per-(g,k,tile) -> block union schedule. A tile's edges are
    # block-sorted; per core a tile spans a few adjacent blocks. The shared
    # schedule must cover the union over cores.
    # tile_blocks[(g,k,t)] = sorted list of bb (0..GBLK-1) with any core edge
    tile_blocks = {}
    per_core = []
    for c in range(n_cores):
        m = ecore == c
        ceg, cek, ced, cep = egrp[m], echunk[m], ed[m], epos[m]
        order = np.lexsort((ced, cep, cek, ceg))  # group, chunk, block(pos), dst
        ceg, cek, ced, cep = ceg[order], cek[order], ced[order], cep[order]
        key = ceg * nchunk + cek
        gk_cnt = np.bincount(key, minlength=NBG * nchunk).reshape(NBG, nchunk)
        starts_flat = np.concatenate([[0], np.cumsum(gk_cnt.reshape(-1))[:-1]])
        starts = starts_flat.reshape(NBG, nchunk)
        per_core.append((ced, cep, gk_cnt, starts))
        for g in range(NBG):
            for k in range(nchunk):
                n = int(gk_cnt[g, k])
                if n == 0:
                    continue
                s0 = int(starts[g, k])
                bb = (cep[s0 : s0 + n] // P - g * GBLK).astype(np.int64)
                for t in range(int(T[g, k])):
                    sl = bb[t * P : (t + 1) * P]
                    if len(sl) == 0:
                        break
                    for b_ in np.unique(sl):
                        tile_blocks.setdefault((g, k, t), set()).add(int(b_))

    # enforce: every non-skip block must appear in >=1 matmul so its PSUM
    # region is written (zero matmul if no core has edges there)
    blocks_seen = set()
    for (g, k, t), bbs in tile_blocks.items():
        for b_ in bbs:
            blocks_seen.add(g * GBLK + b_)
    for blk in range(NBLK_pad):
        if blk in skip_blocks or blk in blocks_seen:
            continue
        g, b_ = blk // GBLK, blk % GBLK
        # attach to first tile of (g, 0) (T[g,0] forced >= 1)
        if T[g, 0] == 0:
            T[g, 0] = 1
        tile_blocks.setdefault((g, 0, 0), set()).add(b_)
    # also ensure groups whose T is all zero but have non-skip blocks get one
    for g in range(NBG):
        blks = [g * GBLK + b_ for b_ in range(GBLK)]
        if all(b_ in skip_blocks for b_ in blks):
            continue
        if T[g].sum() == 0:
            T[g, 0] = 1
            for b_ in range(GBLK):
                if g * GBLK + b_ not in skip_blocks:
                    tile_blocks.setdefault((g, 0, 0), set()).add(b_)

    # seg columns: one per (g,k,t,bb) in global order (g, k, t, bb)
    segcol = {}
    gk_cols = {}  # (g,k) -> (first col, ncol)
    ncols = 0
    for g in range(NBG):
        for k in range(nchunk):
            first = ncols
            for t in range(int(T[g, k])):
                for b_ in sorted(tile_blocks.get((g, k, t), ())):
                    segcol[(g, k, t, b_)] = ncols
                    ncols += 1
            gk_cols[(g, k)] = (first, ncols - first)
    T_TOTAL = ncols

    # per-gather capacities and offsets (idx array laid out per (g,k))
    EG_CAP = T * P  # [NBG, nchunk] idx capacity per (g,k)
    idx_off = np.zeros((NBG, nchunk), dtype=np.int64)
    acc = 0
    for g in range(NBG):
        for k in range(nchunk):
            idx_off[g, k] = acc
            acc += int(EG_CAP[g, k]) // 16
    IDX_COLS = max(int(acc), 16)

    # tile base (column into gbuf) per (g,k): tiles stored [0..T[g,k])
    T_MAX = int(T.max()) if T.size else 1

    # build per-core arrays
    in_maps = []
    pad_real16 = []  # diagnostics: real descs after pad-to-16
    for c in range(n_cores):
        ced, cep, gk_cnt, starts = per_core[c]
        edge_idx_vals = np.full(IDX_COLS * 16, -1, dtype=np.int64)
        seg = np.full((P, max(T_TOTAL, 1)), -1.0, dtype=np.float16)
        ndesc = 0
        for g in range(NBG):
            for k in range(nchunk):
                tcap = int(T[g, k])
                if tcap == 0:
                    continue
                n = int(gk_cnt[g, k])
                s0 = int(starts[g, k])
                base_i = int(idx_off[g, k]) * 16
                # idx values: dst - k*CHUNK, padded to mult of 16 with 0,
                # then -1 (trailing negatives skip descriptor generation)
                if negpad:
                    n16 = _roundup(max(n, 16), 16)
                    vals = np.full(tcap * P, -1, dtype=np.int64)
                else:
                    n16 = tcap * P
                    vals = np.zeros(tcap * P, dtype=np.int64)
                vals[:n] = ced[s0 : s0 + n] - k * CHUNK
                vals[n:n16] = 0
                edge_idx_vals[base_i : base_i + tcap * P] = vals
                ndesc += n16
                # seg values per (tile, block) column: local slot within block
                # for in-block edges, -1 otherwise
                bb_all = cep[s0 : s0 + n] // P - g * GBLK
                loc_all = cep[s0 : s0 + n] % P
                for t in range(tcap):
                    lo, hi = t * P, min((t + 1) * P, n)
                    for b_ in sorted(tile_blocks.get((g, k, t), ())):
                        col = segcol[(g, k, t, b_)]
                        sv = np.full(P, -1.0, dtype=np.float32)
                        if hi > lo:
                            tb = bb_all[lo:hi]
                            tl = loc_all[lo:hi]
                            msk = tb == b_
                            sv[: hi - lo][msk] = tl[msk]
                        seg[:, col] = sv.astype(np.float16)
        assert np.all(edge_idx_vals < CHUNK)
        pad_real16.append(ndesc)

        # self-gather indices (transpose mode: keep pads at 0, no negatives)
        q = np.arange(U_cap)
        self_vals = np.where(
            slot_real[c], slot_node[c] - (q // CSELF_CAP) * CHUNK, 0
        )
        assert np.all(self_vals >= 0) and np.all(self_vals < CHUNK)

        invrow = np.tile(slot_invdeg[c], (P, 1)).astype(np.float32)  # [128, U_cap]

        in_maps.append(
            {
                "features_h": features_h,
                "edge_idx": _wrap16(edge_idx_vals),
                "self_idx": _wrap16(self_vals),
                "seg": seg,
                "invrow": invrow,
                "w1t": W[:, :D].T.astype(np.float16).copy(),
                "w2t": W[:, D:].T.astype(np.float16).copy(),
                "bias_bc": np.tile(b, (P, 1)).astype(np.float32),
                "iota": np.tile(np.arange(P, dtype=np.float16), (P, 1)),
                "ident": np.eye(P, dtype=np.float16),
            }
        )

    # matmul schedule per (g, bb): ordered list of (k, t, segcol)
    sched = {}
    for g in range(NBG):
        for b_ in range(GBLK):
            lst = []
            for k in range(nchunk):
                for t in range(int(T[g, k])):
                    if b_ in tile_blocks.get((g, k, t), ()):
                        lst.append((k, t, segcol[(g, k, t, b_)]))
            sched[(g, b_)] = lst

    plan = {
        "N": N,
        "D": D,
        "nchunk": nchunk,
        "CSELF_CAP": CSELF_CAP,
        "U_cap": U_cap,
        "NBLK": NBLK,
        "NBLK_pad": NBLK_pad,
        "NBG": NBG,
        "T": T,
        "T_MAX": T_MAX,
        "EG_CAP": EG_CAP,
        "idx_off": idx_off,
        "IDX_COLS": IDX_COLS,
        "T_TOTAL": max(T_TOTAL, 1),
        "n_cores": n_cores,
        "bias_nonzero": bool(np.any(b != 0)),
        "skip_blocks": skip_blocks,
        "real_csb": real_csb,
        "sched": sched,
        "gk_cols": gk_cols,
        "pad_real16": pad_real16,
    }

    out_core = core_of[inv]
    out_pos = pos[inv]

    def assemble(core_outputs):
        stacked = np.stack(core_outputs)
        return np.ascontiguousarray(stacked[out_core, out_pos])

    return plan, in_maps, assemble


def build_kernel(plan, reps=1, ge_bufs=16, s_bufs=4, o_bufs=4, p1_bufs=3,
                 p2_bufs=2, ni_tiles=8, nq=4, serialize=False, self_first=True,
                 self_mode="normal", single_packet=True):
    N, D = plan["N"], plan["D"]
    nchunk = plan["nchunk"]
    CSELF_CAP = plan["CSELF_CAP"]
    U_cap = plan["U_cap"]
    NBLK_pad = plan["NBLK_pad"]
    NBG = plan["NBG"]
    T = plan["T"]
    T_MAX = plan["T_MAX"]
    idx_off = plan["idx_off"]
    IDX_COLS = plan["IDX_COLS"]
    T_TOTAL = plan["T_TOTAL"]
    sched = plan["sched"]
    CSB = CSELF_CAP // P
    SMAX = max(n for (_c0, n) in plan["gk_cols"].values())

    f32 = mybir.dt.float32
    f16 = mybir.dt.float16
    NQ = nq
    NI_TILES = ni_tiles
    nc = bacc.Bacc("TRN2", target_bir_lowering=False, num_swdge_queues=NQ)
    cnt_reg = list(
        nc.alloc_registers("gather_cnt", engines=[mybir.EngineType.Pool])
    )[0]
    qrr = [0]
    fence = nc.alloc_semaphore("rep_fence") if serialize else None

    def emit_gather(out3d, tile0, ntiles, in_ap, idx_tile, idxcol0, transpose=False):
        for p0 in range(0, ntiles, NI_TILES):
            p1 = min(p0 + NI_TILES, ntiles)
            ni = (p1 - p0) * P
            nc.gpsimd.reg_mov(cnt_reg, ni)
            if transpose:
                out_ap = out3d[:, :, (tile0 + p0) * P : (tile0 + p1) * P]
            else:
                out_ap = out3d[:, tile0 + p0 : tile0 + p1, :]
            nc.gpsimd.dma_gather(
                out_ap=out_ap,
                in_ap=in_ap,
                idxs_ap=idx_tile[:, idxcol0 + p0 * 8 : idxcol0 + p1 * 8],
                num_idxs=ni,
                num_idxs_reg=cnt_reg,
                elem_size=D,
                transpose=transpose,
                single_packet=single_packet,
                queue_num=qrr[0] % NQ,
            )
            qrr[0] += 1

    feat_h = nc.dram_tensor("features_h", [N, D], f16, kind="ExternalInput")
    edge_idx_d = nc.dram_tensor(
        "edge_idx", [P, IDX_COLS], mybir.dt.int16, kind="ExternalInput"
    )
    self_idx_d = nc.dram_tensor(
        "self_idx", [P, U_cap // 16], mybir.dt.int16, kind="ExternalInput"
    )
    seg_d = nc.dram_tensor("seg", [P, T_TOTAL], f16, kind="ExternalInput")
    invrow_d = nc.dram_tensor("invrow", [P, U_cap], f32, kind="ExternalInput")
    w1t_d = nc.dram_tensor("w1t", [D, D], f16, kind="ExternalInput")
    w2t_d = nc.dram_tensor("w2t", [D, D], f16, kind="ExternalInput")
    bias_d = nc.dram_tensor("bias_bc", [P, D], f32, kind="ExternalInput")
    iota_d = nc.dram_tensor("iota", [P, P], f16, kind="ExternalInput")
    ident_d = nc.dram_tensor("ident", [P, P], f16, kind="ExternalInput")
    out_d = nc.dram_tensor("out", [NBLK_pad * P, D], f32, kind="ExternalOutput")

    with TileContext(nc) as tc:
        with (
            tc.tile_pool(name="const", bufs=1) as const_pool,
            tc.tile_pool(name="gself", bufs=1) as gself_pool,
            tc.tile_pool(name="ge", bufs=ge_bufs) as ge_pool,
            tc.tile_pool(name="s", bufs=s_bufs) as s_pool,
            tc.tile_pool(name="o", bufs=o_bufs) as o_pool,
            tc.tile_pool(name="psum1", bufs=p1_bufs, space="PSUM") as psum1_pool,
            tc.tile_pool(name="psum2", bufs=p2_bufs, space="PSUM") as psum2_pool,
        ):
            nc.gpsimd.load_library(mlp)

            def load_const(dram, shape, dtype=f32, tag=None):
                t = const_pool.tile(shape, dtype, tag=tag)
                nc.sync.dma_start(t[:], dram[:])
                return t

            # split index loads per (g,k)/(chunk) so early gathers don't wait
            # for the whole table
            self_idx = {}
            for k in range(nchunk):
                w = CSELF_CAP // 16
                t = const_pool.tile([P, w], mybir.dt.int16, tag=f"si{k}")
                nc.sync.dma_start(t[:], self_idx_d[:, k * w : (k + 1) * w])
                self_idx[k] = t
            edge_idx = {}
            for g in range(NBG):
                for k in range(nchunk):
                    cap16 = int(T[g, k]) * P // 16
                    if cap16 == 0:
                        continue
                    off = int(idx_off[g, k])
                    t = const_pool.tile(
                        [P, cap16], mybir.dt.int16, tag=f"ei{g}_{k}"
                    )
                    nc.sync.dma_start(t[:], edge_idx_d[:, off : off + cap16])
                    edge_idx[(g, k)] = t
            seg = load_const(seg_d, [P, T_TOTAL], f16, tag="seg")
            invrow = load_const(invrow_d, [P, U_cap], f32, tag="invrow")
            w1t = load_const(w1t_d, [D, D], f16, tag="w1t")
            w2t = load_const(w2t_d, [D, D], f16, tag="w2t")
            bias_bc = load_const(bias_d, [P, D], f32, tag="bias_bc")
            iota = load_const(iota_d, [P, P], f16, tag="iota")
            ident = load_const(ident_d, [P, P], f16, tag="ident")

            for _rep in range(reps):
                if serialize and _rep > 0:
                    nc.gpsimd.wait_ge(fence, 16 * _rep)
                last_dma = [None]

                # self features: transpose mode gives gselfT[f, 0, slot]
                # directly; normal mode gives gself[slot%128, blk, f] and
                # phase 2 PE-transposes per block.
                if self_mode == "transpose":
                    gselfT = gself_pool.tile([P, 1, U_cap], f16)
                else:
                    gself = gself_pool.tile([P, NBLK_pad, D], f16)

                def emit_self():
                    for k in range(nchunk):
                        emit_gather(
                            gselfT if self_mode == "transpose" else gself,
                            k * CSB, int(plan["real_csb"][k]),
                            feat_h[k * CHUNK :, :],
                            self_idx[k], 0,
                            transpose=(self_mode == "transpose"),
                        )
                        # pad runs beyond real_csb: never read (skip blocks)

                if self_first:
                    emit_self()

                for g in range(NBG):
                    gbuf = {}
                    stile = {}
                    for k in range(nchunk):
                        tgk = int(T[g, k])
                        if tgk == 0:
                            continue
                        gb = ge_pool.tile([P, T_MAX, D], f16, tag="ge")
                        emit_gather(
                            gb, 0, tgk,
                            feat_h[k * CHUNK :, :],
                            edge_idx[(g, k)], 0,
                        )
                        gbuf[k] = gb
                        # one-hot tiles for this (g,k)'s seg columns
                        c0, ncol = plan["gk_cols"][(g, k)]
                        if ncol > 0:
                            st = s_pool.tile([P, SMAX, P], f16, tag="s")
                            seg_rep = seg[:, c0 : c0 + ncol].rearrange(
                                "p (t o) -> p t o", o=1
                            ).to_broadcast([P, ncol, P])
                            iota_rep = iota[:, :].rearrange(
                                "p (o w) -> p o w", o=1
                            ).to_broadcast([P, ncol, P])
                            nc.vector.tensor_tensor(
                                out=st[:, :ncol, :],
                                in0=seg_rep,
                                in1=iota_rep,
                                op=mybir.AluOpType.is_equal,
                            )
                            stile[k] = (st, c0)
                    if g == 0 and not self_first:
                        emit_self()

                    psum1 = psum1_pool.tile([P, GBLK, P], f32, tag="p1")
                    for b_ in range(GBLK):
                        lst = sched[(g, b_)]
                        for i, (k, t, sc) in enumerate(lst):
                            st, c0 = stile[k]
                            nc.tensor.matmul(
                                out=psum1[:, b_, :],
                                lhsT=gbuf[k][:, t, :],
                                rhs=st[:, sc - c0, :],
                                start=(i == 0),
                                stop=(i == len(lst) - 1),
                            )

                    for b_ in range(GBLK):
                        blk = g * GBLK + b_
                        if blk in plan["skip_blocks"]:
                            continue
                        if not sched[(g, b_)]:
                            continue
                        msum = o_pool.tile([P, P], f16, tag="msum")
                        nc.vector.tensor_tensor(
                            out=msum[:],
                            in0=psum1[:, b_, :],
                            in1=invrow[:, blk * P : (blk + 1) * P],
                            op=mybir.AluOpType.mult,
                        )
                        if self_mode == "transpose":
                            self_lhsT = gselfT[:, 0, blk * P : (blk + 1) * P]
                        else:
                            psum_tr = psum2_pool.tile([P, P], f32, tag="ptr")
                            nc.tensor.matmul(
                                out=psum_tr[:], lhsT=gself[:, blk, :],
                                rhs=ident[:], start=True, stop=True,
                            )
                            self_t = o_pool.tile([P, P], f16, tag="selfT")
                            nc.scalar.activation(
                                self_t[:], psum_tr[:],
                                mybir.ActivationFunctionType.Copy,
                            )
                            self_lhsT = self_t[:]
                        psum_o = psum2_pool.tile([P, P], f32, tag="po")
                        nc.tensor.matmul(
                            out=psum_o[:], lhsT=msum[:], rhs=w2t[:],
                            start=True, stop=False,
                        )
                        nc.tensor.matmul(
                            out=psum_o[:],
                            lhsT=self_lhsT,
                            rhs=w1t[:],
                            start=False, stop=True,
                        )
                        out_sb = o_pool.tile([P, P], f32, tag="osb")
                        if plan["bias_nonzero"]:
                            o1 = o_pool.tile([P, P], f32, tag="o1")
                            nc.vector.tensor_tensor(
                                out=o1[:], in0=psum_o[:], in1=bias_bc[:],
                                op=mybir.AluOpType.add,
                            )
                            nc.scalar.activation(
                                out_sb[:], o1[:], mybir.ActivationFunctionType.Relu
                            )
                        else:
                            nc.scalar.activation(
                                out_sb[:], psum_o[:],
                                mybir.ActivationFunctionType.Relu,
                            )
                        last_dma[0] = nc.sync.dma_start(
                            out_d[blk * P : (blk + 1) * P, :], out_sb[:]
                        )
                if serialize and last_dma[0] is not None:
                    last_dma[0].then_inc(fence, 16)

    nc.compile()
    return nc


_RUN_KWARGS = {}


def run_on_hw(nc, in_maps, n_cores, **kwargs):
    from concourse.bass_utils import run_bass_kernel_spmd

    return run_bass_kernel_spmd(nc, in_maps, list(range(n_cores)), **kwargs)


def kernel(nodes, features, edge_index, W, b):
    n_cores = 8
    plan, in_maps, assemble = preprocess(
        nodes, features, edge_index, W, b, n_cores=n_cores
    )
    nc = build_kernel(plan)
    res = run_on_hw(nc, in_maps, n_cores, **_RUN_KWARGS)
    outs = [np.asarray(r["out"]) for r in res.results]
    return np.ascontiguousarray(assemble(outs).astype(np.float32))
